# revision 16
# baseline (speedup 1.0000x reference)
"""Bilinear column-correlation head on 8 Trainium2 NeuronCores.

scores[b,i,j] = tanh(x_i^T W x_j + b), diagonal forced to 1.0.
Shapes: x [64, 256, 1024] f32, W [1024, 1024] f32, b scalar f32.
Sharding: data-parallel over batch — each of the 8 cores handles 8 batches.

Per-core schedule (batches processed as 4 pairs, 2 packed per free dim):
  per pair:
    PE-transpose x rows -> xt2[e] = X^T e-slice [128, 512] (f32r)
    phase 1: xwt[d] = sum_e W[e, d-slice]^T @ xt2[e]  (PSUM, 1 cyc/row f32r)
      pair 0: e-outer waves (6 then 2 PSUM banks) so matmuls stream right
      behind W's HBM arrival; later pairs: d-outer, evictions stagger.
    phase 2: pm2 = sum_d xwt[d][:, i-slice]^T @ xt2[d][:, batch-slice]
    ScalarE tanh(psum + bias), affine_select diag := 1.0, DMA out.
All matmul operands are float32r: fp32 rounded to 12 mantissa bits inside
the PE, streaming 1 column/cycle (4x faster than fp32's 4 cyc/row).
"""

import os
import numpy as np

B, N, D = 64, 256, 1024
P = 128
NCORES = 8
B_LOC = B // NCORES          # 8 batches per core
NPAIRS = B_LOC // 2          # batches processed in pairs
DT = D // P                  # 8 contraction tiles
NT = N // P                  # 2 row tiles per batch
W2 = 2 * N                   # 512: packed free dim for a batch pair
HD = D // 2                  # 512: half row width for pair-0 fast start

LAST_RESULTS = None          # BassKernelResults of the most recent run


def _body(ctx, tc, x_ap, w_ap, b_ap, o_ap, matmul_dt, repeat=1):
    from concourse import mybir
    from concourse.masks import make_identity

    nc = tc.nc
    f32 = mybir.dt.float32
    mm = matmul_dt

    const = ctx.enter_context(tc.tile_pool(name="const", bufs=1))
    wpool = ctx.enter_context(tc.tile_pool(name="wpool", bufs=1))
    xrow_pool = ctx.enter_context(tc.tile_pool(name="xrow", bufs=8))
    xt_pool = ctx.enter_context(tc.tile_pool(name="xt", bufs=2 * DT))
    xwt_pool = ctx.enter_context(tc.tile_pool(name="xwt", bufs=12))
    sout_pool = ctx.enter_context(tc.tile_pool(name="sout", bufs=4))
    # pt slots serve both transpose psums [128,512] and phase-2 psums [128,256]
    pt_pool = ctx.enter_context(tc.tile_pool(name="pt", bufs=2, space="PSUM"))
    pm1_pool = ctx.enter_context(tc.tile_pool(name="pm1", bufs=6, space="PSUM"))

    # ---- pair 0: load x rows as half-width tiles so the first transposes
    # start after ~1MB instead of the full 2MB.
    xr0h = []  # [half][k] -> [128, 512] tile
    for half in range(2):
        tiles = []
        for k in range(2 * NT):
            pb, r = divmod(k, NT)
            t = xrow_pool.tile([P, HD], mm, tag="xrh", bufs=8,
                               name=f"xr0_{half}_{k}")
            nc.sync.dma_start(
                t[:],
                x_ap[pb, r * P:(r + 1) * P, half * HD:(half + 1) * HD].bitcast(mm),
            )
            tiles.append(t)
        xr0h.append(tiles)

    identity = const.tile([P, P], mm)
    if mm == f32:
        make_identity(nc, identity)
    else:
        # GPSIMD memset/affine_select can't write f32r: build f32, then copy
        # (the DVE write performs the f32r rounding walrus requires).
        ident_f32 = const.tile([P, P], f32)
        make_identity(nc, ident_f32)
        nc.vector.tensor_copy(identity[:], ident_f32[:])
    bias_sb = const.tile([P, 1], f32)
    nc.sync.dma_start(bias_sb[0:1, 0:1], b_ap[:, :])
    nc.gpsimd.partition_broadcast(bias_sb[:, 0:1], bias_sb[0:1, 0:1])

    # W resident in SBUF for the whole kernel (f32r view of the raw bits)
    w_sb = []
    for e in range(DT):
        wt = wpool.tile([P, D], mm, tag=f"w{e}")
        nc.sync.dma_start(wt[:], w_ap[e * P:(e + 1) * P, :].bitcast(mm))
        w_sb.append(wt)

    def load_pair_rows(bp):
        b0 = 2 * bp
        xr = []
        for k in range(2 * NT):
            pb, r = divmod(k, NT)
            t = xrow_pool.tile([P, D], mm, tag="xr")
            nc.sync.dma_start(
                t[:], x_ap[b0 + pb, r * P:(r + 1) * P, :].bitcast(mm)
            )
            xr.append(t)
        return xr

    xr_next = None
    WAVES = (6, 2)  # first-pair phase-1 PSUM groups per wave

    steps = [(rep, bp) for rep in range(repeat) for bp in range(NPAIRS)]
    for idx, (rep, bp) in enumerate(steps):
        first = idx == 0
        b0 = 2 * bp
        xr = xr_next
        uid = f"{rep}_{bp}"

        # ---- transposes: xt2[e][:, k*128:(k+1)*128] = xr[k][:, e-slice].T
        xt2 = []
        for e in range(DT):
            pt = pt_pool.tile([P, W2], mm, tag="pt", name=f"pt_{uid}_{e}")
            for k in range(2 * NT):
                if first:
                    src = xr0h[e // (DT // 2)][k][:, (e % (DT // 2)) * P:
                                                  (e % (DT // 2) + 1) * P]
                else:
                    src = xr[k][:, e * P:(e + 1) * P]
                nc.tensor.transpose(pt[:, k * P:(k + 1) * P], src, identity[:])
            xt = xt_pool.tile([P, W2], mm, tag="xt", name=f"xt_{uid}_{e}")
            nc.vector.tensor_copy(xt[:], pt[:])
            xt2.append(xt)

        # prefetch next step's rows while phase 1 runs
        if idx + 1 < len(steps):
            xr_next = load_pair_rows(steps[idx + 1][1])

        xwt = [None] * DT
        if first:
            # W still streaming from HBM: e-outer waves consume each W e-tile
            # the moment it lands.
            d0 = 0
            for wave, width in enumerate(WAVES):
                pms = [
                    pm1_pool.tile([P, W2], f32, tag="pm1",
                                  name=f"pm1_{uid}_{wave}_{i}")
                    for i in range(width)
                ]
                for e in range(DT):
                    for i in range(width):
                        d = d0 + i
                        nc.tensor.matmul(
                            pms[i][:],
                            lhsT=w_sb[e][:, d * P:(d + 1) * P],
                            rhs=xt2[e][:],
                            start=(e == 0),
                            stop=(e == DT - 1),
                        )
                for i in range(width):
                    t = xwt_pool.tile([P, W2], mm, tag="xwt",
                                      name=f"xwt_{uid}_{d0 + i}")
                    nc.scalar.copy(t[:], pms[i][:])
                    xwt[d0 + i] = t
                d0 += width
        else:
            # Steady state: d-outer groups pipeline through 6 PSUM slots.
            for d in range(DT):
                pm = pm1_pool.tile([P, W2], f32, tag="pm1",
                                   name=f"pm1_{uid}_{d}")
                for e in range(DT):
                    nc.tensor.matmul(
                        pm[:],
                        lhsT=w_sb[e][:, d * P:(d + 1) * P],
                        rhs=xt2[e][:],
                        start=(e == 0),
                        stop=(e == DT - 1),
                    )
                t = xwt_pool.tile([P, W2], mm, tag="xwt",
                                  name=f"xwt_{uid}_{d}")
                nc.scalar.copy(t[:], pm[:])
                xwt[d] = t

        # ---- phase 2 + tanh + diag + store
        for pb in range(2):
            for it in range(NT):
                pm2 = pt_pool.tile([P, N], f32, tag="pt",
                                   name=f"pm2_{uid}_{pb}_{it}")
                for d in range(DT):
                    nc.tensor.matmul(
                        pm2[:],
                        lhsT=xwt[d][:, pb * N + it * P: pb * N + (it + 1) * P],
                        rhs=xt2[d][:, pb * N:(pb + 1) * N],
                        start=(d == 0),
                        stop=(d == DT - 1),
                    )
                s_sb = sout_pool.tile([P, N], f32, tag="s",
                                      name=f"s_{uid}_{pb}_{it}")
                nc.scalar.activation(
                    s_sb[:], pm2[:], mybir.ActivationFunctionType.Tanh,
                    bias=bias_sb[:, 0:1],
                )
                nc.gpsimd.affine_select(
                    out=s_sb[:],
                    in_=s_sb[:],
                    compare_op=mybir.AluOpType.not_equal,
                    fill=1.0,
                    base=it * P,
                    pattern=[[-1, N]],
                    channel_multiplier=1,
                )
                nc.sync.dma_start(
                    o_ap[b0 + pb, it * P:(it + 1) * P, :], s_sb[:]
                )


def kernel(column_embeddings, W, b):
    global LAST_RESULTS
    import concourse.tile as tile
    from concourse import bacc, mybir
    from concourse.bass_utils import run_bass_kernel_spmd
    from contextlib import ExitStack

    x = np.ascontiguousarray(np.asarray(column_embeddings, dtype=np.float32))
    w = np.ascontiguousarray(np.asarray(W, dtype=np.float32))
    bias = np.asarray(b, dtype=np.float32).reshape(1, 1)

    matmul_dt = (
        mybir.dt.float32r
        if os.environ.get("KERNEL_MM_DT", "f32r") == "f32r"
        else mybir.dt.float32
    )

    nc = bacc.Bacc(
        "TRN2", target_bir_lowering=False, debug=False, num_devices=NCORES
    )
    x_t = nc.dram_tensor("x", [B_LOC, N, D], mybir.dt.float32, kind="ExternalInput")
    w_t = nc.dram_tensor("w", [D, D], mybir.dt.float32, kind="ExternalInput")
    b_t = nc.dram_tensor("bias", [1, 1], mybir.dt.float32, kind="ExternalInput")
    o_t = nc.dram_tensor("out", [B_LOC, N, N], mybir.dt.float32, kind="ExternalOutput")

    repeat = int(os.environ.get("KERNEL_REPEAT", "1"))
    with tile.TileContext(nc) as tc, ExitStack() as ctx:
        _body(ctx, tc, x_t.ap(), w_t.ap(), b_t.ap(), o_t.ap(), matmul_dt,
              repeat=repeat)
    nc.compile()

    in_maps = [
        {"x": x[c * B_LOC:(c + 1) * B_LOC], "w": w, "bias": bias}
        for c in range(NCORES)
    ]
    res = run_bass_kernel_spmd(
        nc,
        in_maps,
        list(range(NCORES)),
        trace=bool(os.environ.get("KERNEL_TRACE")),
    )
    LAST_RESULTS = res
    out = np.concatenate(
        [res.results[c]["out"] for c in range(NCORES)], axis=0
    )
    return out
